# revision 5
# baseline (speedup 1.0000x reference)
"""Masked single-head attention on 8 TRN2 NeuronCores.

Problem: q,k,v [8, 2048, 128] f32, event_lengths [8] i32.
  scores = q @ k^T / sqrt(128); masked (i<len & j<len) else -1e9; softmax; @ v.

Sharding: data-parallel over batch — core b computes batch element b's full
2048x2048 attention.

Per-core algorithm (transposed-scores layout, no max-subtraction):
  S^T[j,i] = sum_d kT[d,j] * qT[d,i]            (bf16 matmuls, N=512)
  E[j,i]   = exp(S^T/sqrt(128) + akv[j])        (ACT; akv[j] = 0 / -1e9 kv mask,
                                                 applied free as per-partition bias)
  o^T[d,i] = sum_j v[j,d] * E[j,i]              (bf16 matmuls, accumulated in PSUM)
  sum[i]   = sum_j E[j,i]                       (ones-vector matmul)
  out[i,d] = o^T.T * (mq[i]/sum[i]) + meanV[d] * (1-mq[i])
             (PE transpose + per-partition scalars; mq[i] is the query mask,
              rows i>=len get mean(V) exactly like the reference's uniform
              softmax over an all -1e9 row)

Scores are bounded (|s| < ~20) so exp() cannot overflow in f32 and skipping the
max-subtraction is exact up to normal fp rounding.
"""

import numpy as np

HEAD_DIM = 128
B, S = 8, 2048
NEG = -1e9
P = 128
NCHUNK = S // P  # 16 j-chunks
BLKW = 1024  # query block width (2 PSUM banks wide)
NBLK = S // BLKW  # 2
HB = BLKW // 512  # 512-wide matmul halves per block (2)
NCOL = BLKW // P  # 128-cols per block (8)
INV_SQRT_D = 1.0 / np.sqrt(HEAD_DIM)
EPS = 1e-30

_cache = {}
_last_in_maps = None


def _build():
    import concourse.tile as tile
    from concourse import bacc, mybir

    F32 = mybir.dt.float32
    BF16 = mybir.dt.bfloat16
    EXP = mybir.ActivationFunctionType.Exp

    nc = bacc.Bacc("TRN2", target_bir_lowering=False, debug=False, num_devices=8)

    qT = nc.declare_dram_parameter("qT", [P, S], BF16, isOutput=False)
    kT = nc.declare_dram_parameter("kT", [P, S], BF16, isOutput=False)
    v = nc.declare_dram_parameter("v", [S, P], BF16, isOutput=False)
    akvT = nc.declare_dram_parameter("akvT", [P, NCHUNK], F32, isOutput=False)
    mqT = nc.declare_dram_parameter("mqT", [P, S // P], F32, isOutput=False)
    nmqT = nc.declare_dram_parameter("nmqT", [P, S // P], F32, isOutput=False)
    onesc = nc.declare_dram_parameter("onesc", [P, 1], BF16, isOutput=False)
    meanvb = nc.declare_dram_parameter("meanvb", [P, P], F32, isOutput=False)
    identd = nc.declare_dram_parameter("identd", [P, P], F32, isOutput=False)
    out = nc.declare_dram_parameter("out", [S, P], F32, isOutput=True)

    with tile.TileContext(nc) as tc:
        with (
            tc.tile_pool(name="const", bufs=1) as const,
            tc.tile_pool(name="qk", bufs=1) as qk,
            tc.tile_pool(name="vp", bufs=1) as vp,
            tc.tile_pool(name="e", bufs=6) as epool,
            tc.tile_pool(name="stage", bufs=2) as stage,
            tc.tile_pool(name="fins", bufs=4) as fins,
            tc.tile_pool(name="ps_s", bufs=2, space="PSUM") as ps_s,
            tc.tile_pool(name="ps_o", bufs=1, space="PSUM") as ps_o,
            tc.tile_pool(name="ps_sum", bufs=1, space="PSUM") as ps_sum,
        ):
            # ---- constants / resident inputs ----
            akv_t = const.tile([P, NCHUNK], F32, tag="akv")
            nc.sync.dma_start(akv_t[:], akvT[:, :])
            mq_t = const.tile([P, S // P], F32, tag="mq")
            nc.sync.dma_start(mq_t[:], mqT[:, :])
            nmq_t = const.tile([P, S // P], F32, tag="nmq")
            nc.sync.dma_start(nmq_t[:], nmqT[:, :])
            ones_t = const.tile([P, 1], BF16, tag="ones")
            nc.sync.dma_start(ones_t[:], onesc[:, :])
            meanv_t = const.tile([P, P], F32, tag="meanv")
            nc.sync.dma_start(meanv_t[:], meanvb[:, :])
            ident_t = const.tile([P, P], F32, tag="ident")
            nc.sync.dma_start(ident_t[:], identd[:, :])

            # kT resident, loaded per chunk so the first matmuls start early
            kT_t = qk.tile([P, S], BF16, tag="kT")
            for jc in range(NCHUNK):
                nc.sync.dma_start(kT_t[:, jc * P : (jc + 1) * P], kT[:, jc * P : (jc + 1) * P])
            # qT resident, loaded per block
            qT_t = qk.tile([P, S], BF16, tag="qT")
            for ib in range(NBLK):
                nc.sync.dma_start(
                    qT_t[:, ib * BLKW : (ib + 1) * BLKW], qT[:, ib * BLKW : (ib + 1) * BLKW]
                )
            # v resident as 16 chunks [128, 128]
            v_t = vp.tile([P, NCHUNK * P], BF16, tag="v")
            for jc in range(NCHUNK):
                nc.sync.dma_start(v_t[:, jc * P : (jc + 1) * P], v[jc * P : (jc + 1) * P, :])

            for ib in range(NBLK):
                q0 = ib * BLKW
                po = ps_o.tile([P, BLKW], F32, tag="o")
                psm0 = ps_sum.tile([1, 512], F32, tag="sum0")
                psm1 = ps_sum.tile([1, 512], F32, tag="sum1")
                psm = [psm0, psm1]

                # software-pipelined over j-chunks: S(jc) runs 2 ahead of
                # AV/sum(jc) so ACT's exp latency is hidden behind PE work.
                LOOKAHEAD = 2
                es = [None] * NCHUNK

                def emit_s(jc):
                    ps = ps_s.tile([P, BLKW], F32, tag="s")
                    for h in range(HB):
                        nc.tensor.matmul(
                            ps[:, h * 512 : (h + 1) * 512],
                            kT_t[:, jc * P : (jc + 1) * P],
                            qT_t[:, q0 + h * 512 : q0 + (h + 1) * 512],
                            start=True,
                            stop=True,
                        )
                    e = epool.tile([P, BLKW], BF16, tag="e")
                    nc.scalar.activation(
                        e[:], ps[:], EXP, bias=akv_t[:, jc : jc + 1], scale=INV_SQRT_D
                    )
                    es[jc] = e

                def emit_av(jc):
                    e = es[jc]
                    for h in range(HB):
                        nc.tensor.matmul(
                            po[:, h * 512 : (h + 1) * 512],
                            v_t[:, jc * P : (jc + 1) * P],
                            e[:, h * 512 : (h + 1) * 512],
                            start=(jc == 0),
                            stop=(jc == NCHUNK - 1),
                        )
                    for h in range(HB):
                        nc.tensor.matmul(
                            psm[h][:],
                            ones_t[:],
                            e[:, h * 512 : (h + 1) * 512],
                            start=(jc == 0),
                            stop=(jc == NCHUNK - 1),
                        )

                for jc in range(LOOKAHEAD):
                    emit_s(jc)
                for jc in range(NCHUNK):
                    if jc + LOOKAHEAD < NCHUNK:
                        emit_s(jc + LOOKAHEAD)
                    emit_av(jc)

                # copy o^T and sums to SBUF
                ot = stage.tile([P, BLKW], F32, tag="ot")
                nc.vector.tensor_copy(ot[:], po[:])
                sums = stage.tile([1, BLKW], F32, tag="sums")
                nc.vector.tensor_copy(sums[:, 0:512], psm[0][:])
                nc.vector.tensor_copy(sums[:, 512:1024], psm[1][:])

                # transpose sums [1,1024] -> [128,8] via tiny matmuls with [[1.0]]
                prs = ps_sum.tile([P, NCOL], F32, tag="sum0")
                for c in range(NCOL):
                    nc.tensor.matmul(
                        prs[:, c : c + 1],
                        sums[0:1, c * P : (c + 1) * P],
                        ident_t[0:1, 0:1],
                        start=True,
                        stop=True,
                    )
                # r = 1 / (sums + eps), then a = r * mq (both [128,NCOL])
                rs = stage.tile([P, NCOL], F32, tag="rs")
                nc.vector.tensor_scalar(rs[:], prs[:], EPS, None, mybir.AluOpType.add)
                nc.vector.reciprocal(rs[:], rs[:])
                a_t = stage.tile([P, NCOL], F32, tag="a")
                nc.vector.tensor_tensor(
                    a_t[:],
                    rs[:],
                    mq_t[:, ib * NCOL : (ib + 1) * NCOL],
                    mybir.AluOpType.mult,
                )

                # transpose o^T blocks to [i,d], normalize, blend meanV, DMA out
                pt = ps_o.tile([P, BLKW], F32, tag="o")
                for c in range(NCOL):
                    nc.tensor.transpose(
                        pt[:, c * P : (c + 1) * P], ot[:, c * P : (c + 1) * P], ident_t[:]
                    )
                for c in range(NCOL):
                    col = ib * NCOL + c
                    fin = fins.tile([P, P], F32, tag="fin")
                    nc.vector.tensor_scalar(
                        fin[:],
                        pt[:, c * P : (c + 1) * P],
                        a_t[:, c : c + 1],
                        None,
                        mybir.AluOpType.mult,
                    )
                    mterm = fins.tile([P, P], F32, tag="mterm")
                    nc.vector.tensor_scalar(
                        mterm[:],
                        meanv_t[:],
                        nmq_t[:, col : col + 1],
                        None,
                        mybir.AluOpType.mult,
                    )
                    nc.vector.tensor_tensor(fin[:], fin[:], mterm[:], mybir.AluOpType.add)
                    nc.sync.dma_start(out[col * P : (col + 1) * P, :], fin[:])

    nc.compile()
    return nc


def _get_nc():
    if "nc" not in _cache:
        _cache["nc"] = _build()
    return _cache["nc"]


def _prep_in_maps(q, k, v, lens):
    import ml_dtypes

    bf16 = ml_dtypes.bfloat16
    ident = np.eye(P, dtype=np.float32)
    onesc = np.ones((P, 1), bf16)
    j_idx = np.arange(S)

    in_maps = []
    for b in range(B):
        ln = int(lens[b])
        akv = np.where(j_idx < ln, 0.0, NEG).astype(np.float32)
        mq = (j_idx < ln).astype(np.float32)
        in_maps.append(
            {
                "qT": np.ascontiguousarray(q[b].T).astype(bf16),
                "kT": np.ascontiguousarray(k[b].T).astype(bf16),
                "v": v[b].astype(bf16),
                "akvT": np.ascontiguousarray(akv.reshape(NCHUNK, P).T),
                "mqT": np.ascontiguousarray(mq.reshape(S // P, P).T),
                "nmqT": np.ascontiguousarray((1.0 - mq).reshape(S // P, P).T),
                "onesc": onesc,
                "meanvb": np.broadcast_to(
                    v[b].mean(axis=0, dtype=np.float64).astype(np.float32), (P, P)
                ).copy(),
                "identd": ident,
            }
        )
    return in_maps


def kernel(q, k, v, event_lengths):
    q = np.asarray(q, dtype=np.float32)
    k = np.asarray(k, dtype=np.float32)
    v = np.asarray(v, dtype=np.float32)
    lens = np.asarray(event_lengths).astype(np.int64)

    nc = _get_nc()
    in_maps = _prep_in_maps(q, k, v, lens)

    global _last_in_maps
    _last_in_maps = in_maps

    from concourse.bass_utils import run_bass_kernel_spmd

    res = run_bass_kernel_spmd(nc, in_maps, core_ids=list(range(B)))
    out = np.stack([res.results[b]["out"] for b in range(B)], axis=0)
    return out


if __name__ == "__main__":
    rng = np.random.default_rng(0)
    q = rng.standard_normal((B, S, HEAD_DIM)).astype(np.float32)
    k = rng.standard_normal((B, S, HEAD_DIM)).astype(np.float32)
    v_ = rng.standard_normal((B, S, HEAD_DIM)).astype(np.float32)
    lens = rng.integers(0, S, size=(B,)).astype(np.int32)
    o = kernel(q=q, k=k, v=v_, event_lengths=lens)
    print(o.shape, o.dtype)
